# revision 6
# baseline (speedup 1.0000x reference)
"""MoE LoRA adapter layer (top-2 routed, E=8 experts, R=16) on 8 TRN2 NeuronCores.

Strategy: data-parallel over batch B=32 -> 4 batches/core; router + LoRA
weights replicated (tiny). Key observation: E*R = 128 = partition width, so
the per-expert LoRA down/up projections stack into two dense matmuls with the
expert sum folded into the contraction; gates (exactly 0 off the top-2) are
applied by scaling P1 columns.

v2: x is shipped PRE-TRANSPOSED from the host as xT [H, T] bf16, so
  MM1:  p1[er, t]  = sum_h D_all[er, h] * xT[h, t]     (lhsT = D^T tiles)
  MM2:  yT[h, t]   = xT[h, t] + sum_er U_all[er, h] * g*p1[er, t]
needs ZERO on-device transposes: MM2's lhsT is U_all in its natural [er, h]
layout and the output is produced directly in the same transposed layout as
the input, so the residual add reuses the resident xT tiles. The host
transposes the output back. This removes the 128 PE transposes + full-tensor
Scalar copies of v1 and lets stores stream k-tile by k-tile.

DMA: 8 x-tile loads of 512 KiB (4 KiB/partition contiguous runs) on the
SWDGE/gpsimd queue; weights on sync; stores alternate scalar/sync HWDGE
queues. MM1 runs k-major so it streams behind the loads; all four batch
chunks' PSUM accumulators live simultaneously (4 banks) + 4 banks for MM2.
"""

import sys

if "/opt/trn_rl_repo" not in sys.path:
    sys.path.insert(0, "/opt/trn_rl_repo")

import numpy as np
import ml_dtypes

import concourse.bass as bass
import concourse.tile as tile
from concourse import bacc, mybir
from concourse.bass_utils import run_bass_kernel_spmd

B, L, H = 32, 512, 1024
E, R, TOP_K = 8, 16, 2
N_CORES = 8
NB = B // N_CORES          # batches per core = 4
T = NB * L                 # tokens per core = 2048
P = 128                    # partitions
NK = H // P                # H k-tiles = 8

F32 = mybir.dt.float32
BF16 = mybir.dt.bfloat16
BF16_NP = ml_dtypes.bfloat16

_COMPILED = None


def _build():
    """Build + compile the single-core program (same on all 8 cores)."""
    nc = bacc.Bacc("TRN2", target_bir_lowering=False, debug=False)

    xt_in = nc.dram_tensor("xt_in", [H, T], BF16, kind="ExternalInput")
    cls_in = nc.dram_tensor("cls_in", [NB, H], F32, kind="ExternalInput")
    d_t = nc.dram_tensor("d_t", [P, NK * P], BF16, kind="ExternalInput")
    u_in = nc.dram_tensor("u_in", [P, H], BF16, kind="ExternalInput")
    rwt = nc.dram_tensor("rwt", [P, NK * E], F32, kind="ExternalInput")
    rep = nc.dram_tensor("rep", [E, P], F32, kind="ExternalInput")
    idn = nc.dram_tensor("idn", [NB, NB], F32, kind="ExternalInput")
    y_out = nc.dram_tensor("y_out", [H, T], BF16, kind="ExternalOutput")

    x_k = xt_in.ap().rearrange("(k p) t -> k p t", p=P)
    y_k = y_out.ap().rearrange("(k p) t -> k p t", p=P)

    with tile.TileContext(nc) as tc:
        with (
            tc.tile_pool(name="wpool", bufs=1) as wpool,
            tc.tile_pool(name="xpool", bufs=1) as xpool,
            tc.tile_pool(name="ypool", bufs=1) as ypool,
            tc.tile_pool(name="p2pool", bufs=1) as p2pool,
            tc.tile_pool(name="gpool", bufs=1) as gpool,
            tc.tile_pool(name="p1_ps", bufs=1, space="PSUM") as p1_ps,
            tc.tile_pool(name="w_ps", bufs=4, space="PSUM") as w_ps,
        ):
            # ---- x loads first so bytes flow ASAP (SWDGE/gpsimd queue) ----
            xk = []
            for k in range(NK):
                xt = xpool.tile([P, T], BF16, tag=f"x{k}")
                nc.gpsimd.dma_start(xt[:], x_k[k])
                xk.append(xt)

            # ---- small constants on the sync HWDGE queue ----
            cls_nat = gpool.tile([NB, H], F32, tag="cls")
            nc.sync.dma_start(cls_nat[:], cls_in.ap())
            id_sb = wpool.tile([NB, NB], F32, tag="idn")
            nc.sync.dma_start(id_sb[:], idn.ap())
            rwt_sb = wpool.tile([P, NK * E], F32, tag="rwt")
            nc.sync.dma_start(rwt_sb[:], rwt.ap())
            rep_sb = wpool.tile([E, P], F32, tag="rep")
            nc.sync.dma_start(rep_sb[:], rep.ap())
            d_sb = wpool.tile([P, NK * P], BF16, tag="d")
            nc.sync.dma_start(d_sb[:], d_t.ap())
            u_sb = wpool.tile([P, H], BF16, tag="u")
            nc.sync.dma_start(u_sb[:], u_in.ap())

            # ---- gates prologue: clean logits -> exact top-2 softmax ----
            clsT = gpool.tile([P, NK * NB], F32, tag="clsT")
            cps = w_ps.tile([P, L], F32, tag="w")
            for k in range(NK):
                nc.tensor.transpose(
                    cps[:, k * NB : (k + 1) * NB],
                    cls_nat[:, k * P : (k + 1) * P],
                    id_sb[:],
                )
            nc.vector.tensor_copy(clsT[:, 0 : NK * NB], cps[:, 0 : NK * NB])

            lg_ps = w_ps.tile([P, L], F32, tag="w")
            for k in range(NK):
                nc.tensor.matmul(
                    lg_ps[0:NB, 0:E],
                    clsT[:, k * NB : (k + 1) * NB],
                    rwt_sb[:, k * E : (k + 1) * E],
                    start=(k == 0),
                    stop=(k == NK - 1),
                )
            lg = gpool.tile([NB, E], F32, tag="lg")
            nc.vector.tensor_copy(lg[:], lg_ps[0:NB, 0:E])

            # top-2 softmax per row (E=8 along free dim)
            m1 = gpool.tile([NB, 1], F32, tag="m1")
            nc.vector.reduce_max(m1[:], lg[:], axis=mybir.AxisListType.X)
            t_sb = gpool.tile([NB, E], F32, tag="t")
            nc.vector.tensor_scalar(
                t_sb[:], lg[:], m1[:], None, op0=mybir.AluOpType.subtract
            )
            # pen = (t >= 0) * 1e30  (knocks out the argmax)
            pen = gpool.tile([NB, E], F32, tag="pen")
            nc.vector.tensor_scalar(
                pen[:], t_sb[:], 0.0, 1e30,
                op0=mybir.AluOpType.is_ge, op1=mybir.AluOpType.mult,
            )
            t2 = gpool.tile([NB, E], F32, tag="t2")
            nc.vector.tensor_sub(t2[:], t_sb[:], pen[:])
            m2 = gpool.tile([NB, 1], F32, tag="m2")
            nc.vector.reduce_max(m2[:], t2[:], axis=mybir.AxisListType.X)
            keep = gpool.tile([NB, E], F32, tag="keep")
            nc.vector.tensor_scalar(
                keep[:], t_sb[:], m2[:], None, op0=mybir.AluOpType.is_ge
            )
            ex = gpool.tile([NB, E], F32, tag="ex")
            nc.scalar.activation(ex[:], t_sb[:], mybir.ActivationFunctionType.Exp)
            eg = gpool.tile([NB, E], F32, tag="eg")
            nc.vector.tensor_mul(eg[:], ex[:], keep[:])
            s_sb = gpool.tile([NB, 1], F32, tag="s")
            nc.vector.reduce_sum(s_sb[:], eg[:], axis=mybir.AxisListType.X)
            rs = gpool.tile([NB, 1], F32, tag="rs")
            nc.vector.reciprocal(rs[:], s_sb[:])
            gts = gpool.tile([NB, E], F32, tag="gts")
            nc.vector.tensor_scalar(
                gts[:], eg[:], rs[:], None, op0=mybir.AluOpType.mult
            )

            # gatesT then replicate x16 along partitions -> gvec [128, NB]
            gt_ps = w_ps.tile([P, L], F32, tag="w")
            nc.tensor.transpose(gt_ps[0:E, 0:NB], gts[:], id_sb[:])
            gtT = gpool.tile([E, NB], F32, tag="gtT")
            nc.vector.tensor_copy(gtT[:], gt_ps[0:E, 0:NB])
            gv_ps = w_ps.tile([P, L], F32, tag="w")
            nc.tensor.matmul(gv_ps[:, 0:NB], rep_sb[:], gtT[:])
            gvec = gpool.tile([P, NB], F32, tag="gvec")
            nc.vector.tensor_copy(gvec[:], gv_ps[:, 0:NB])

            # ---- MM1, k-major so it streams behind the x loads ----
            p1 = [
                p1_ps.tile([P, L], F32, tag=f"p1_{c}", name=f"p1_{c}")
                for c in range(NB)
            ]
            for k in range(NK):
                for c in range(NB):
                    nc.tensor.matmul(
                        p1[c][:],
                        d_sb[:, k * P : (k + 1) * P],
                        xk[k][:, c * L : (c + 1) * L],
                        start=(k == 0),
                        stop=(k == NK - 1),
                    )

            # gate scaling: p2[er, t] = gvec[er, c] * p1[er, t]
            p2 = []
            for c in range(NB):
                p2t = p2pool.tile([P, L], BF16, tag=f"p2_{c}")
                nc.vector.tensor_scalar(
                    p2t[:], p1[c][:], gvec[:, c : c + 1], None,
                    op0=mybir.AluOpType.mult,
                )
                p2.append(p2t)

            # ---- MM2 + residual, k-major; store each finished yT k-tile ----
            for k in range(NK):
                yt = ypool.tile([P, T], BF16, tag=f"y{k}")
                for c in range(NB):
                    wp = w_ps.tile([P, L], F32, tag="w")
                    nc.tensor.matmul(
                        wp[:],
                        u_sb[:, k * P : (k + 1) * P],
                        p2[c][:],
                    )
                    nc.vector.tensor_add(
                        yt[:, c * L : (c + 1) * L],
                        wp[:],
                        xk[k][:, c * L : (c + 1) * L],
                    )
                eng = nc.scalar if k % 2 == 0 else nc.sync
                eng.dma_start(y_k[k], yt[:])

    nc.compile()
    return nc


def _weights_maps(router_w, lora_down, lora_up):
    # D_all[(e,r), h] stacked; lhsT tiles need [p, k, m] = D_all[m, k*128+p]
    d_all = lora_down.reshape(E * R, H)                       # [128, 1024]
    d_t = np.ascontiguousarray(
        d_all.T.reshape(NK, P, E * R).transpose(1, 0, 2).reshape(P, NK * P)
    ).astype(BF16_NP)
    # U_all[(e,r), h] = lora_up[e, h, r]
    u_np = np.ascontiguousarray(
        lora_up.transpose(0, 2, 1).reshape(E * R, H)
    ).astype(BF16_NP)
    # router_wT tiles [p, k, e] = router_w[e, k*128+p]
    rwt_np = np.ascontiguousarray(
        router_w.T.reshape(NK, P, E).transpose(1, 0, 2).reshape(P, NK * E)
    ).astype(np.float32)
    rep_np = np.zeros((E, P), np.float32)
    for e in range(E):
        rep_np[e, e * R : (e + 1) * R] = 1.0
    idn_np = np.eye(NB, dtype=np.float32)
    return {
        "d_t": d_t, "u_in": u_np, "rwt": rwt_np, "rep": rep_np,
        "idn": idn_np,
    }


def get_compiled():
    global _COMPILED
    if _COMPILED is None:
        _COMPILED = _build()
    return _COMPILED


def make_in_maps(x, router_w, lora_down, lora_up):
    x = np.asarray(x, np.float32)
    w_maps = _weights_maps(
        np.asarray(router_w, np.float32),
        np.asarray(lora_down, np.float32),
        np.asarray(lora_up, np.float32),
    )
    in_maps = []
    for i in range(N_CORES):
        shard = x[i * NB : (i + 1) * NB]                       # [NB, L, H]
        xt = np.ascontiguousarray(
            shard.reshape(T, H).T
        ).astype(BF16_NP)                                      # [H, T]
        cls_shard = np.ascontiguousarray(shard[:, 0, :])
        in_maps.append({"xt_in": xt, "cls_in": cls_shard, **w_maps})
    return in_maps


def unshard_one(yt):
    """Per-core device output [H, T] -> [NB, L, H] float32."""
    return np.asarray(yt, np.float32).T.reshape(NB, L, H)


def kernel(x, router_w, lora_down, lora_up):
    nc = get_compiled()
    in_maps = make_in_maps(x, router_w, lora_down, lora_up)
    res = run_bass_kernel_spmd(nc, in_maps, core_ids=list(range(N_CORES)))
    out = np.empty((B, L, H), np.float32)
    for i in range(N_CORES):
        out[i * NB : (i + 1) * NB] = unshard_one(res.results[i]["y_out"])
    return out


# revision 13
# speedup vs baseline: 1.1495x; 1.1495x over previous
"""MoE LoRA adapter layer (top-2 routed, E=8 experts, R=16) on 8 TRN2 NeuronCores.

Strategy: data-parallel over batch B=32 -> 4 batches/core; router + LoRA
weights replicated (tiny). Key observation: E*R = 128 = partition width, so
the per-expert LoRA down/up projections stack into two dense matmuls with the
expert sum folded into the contraction; gates (exactly 0 off the top-2) are
applied by scaling P1 columns.

v2: x is shipped PRE-TRANSPOSED from the host as xT [H, T] bf16, so
  MM1:  p1[er, t]  = sum_h D_all[er, h] * xT[h, t]     (lhsT = D^T tiles)
  MM2:  yT[h, t]   = xT[h, t] + sum_er U_all[er, h] * g*p1[er, t]
needs ZERO on-device transposes: MM2's lhsT is U_all in its natural [er, h]
layout and the output is produced directly in the same transposed layout as
the input, so the residual add reuses the resident xT tiles. The host
transposes the output back. This removes the 128 PE transposes + full-tensor
Scalar copies of v1 and lets stores stream k-tile by k-tile.

DMA: 8 x-tile loads of 512 KiB (4 KiB/partition contiguous runs) on the
SWDGE/gpsimd queue; weights on sync; stores alternate scalar/sync HWDGE
queues. MM1 runs k-major so it streams behind the loads; all four batch
chunks' PSUM accumulators live simultaneously (4 banks) + 4 banks for MM2.
"""

import sys

if "/opt/trn_rl_repo" not in sys.path:
    sys.path.insert(0, "/opt/trn_rl_repo")

import numpy as np
import ml_dtypes

import concourse.bass as bass
import concourse.tile as tile
from concourse import bacc, mybir
from concourse.bass_utils import run_bass_kernel_spmd

B, L, H = 32, 512, 1024
E, R, TOP_K = 8, 16, 2
N_CORES = 8
NB = B // N_CORES          # batches per core = 4
T = NB * L                 # tokens per core = 2048
P = 128                    # partitions
NK = H // P                # H k-tiles = 8

F32 = mybir.dt.float32
BF16 = mybir.dt.bfloat16
BF16_NP = ml_dtypes.bfloat16

_COMPILED = None


def _build():
    """Build + compile the single-core program (same on all 8 cores)."""
    nc = bacc.Bacc("TRN2", target_bir_lowering=False, debug=False)

    xt_in = nc.dram_tensor("xt_in", [H, T], BF16, kind="ExternalInput")
    cls_in = nc.dram_tensor("cls_in", [NB, H], F32, kind="ExternalInput")
    d_t = nc.dram_tensor("d_t", [P, NK * P], BF16, kind="ExternalInput")
    u_in = nc.dram_tensor("u_in", [P, H], BF16, kind="ExternalInput")
    rwt = nc.dram_tensor("rwt", [P, NK * E], F32, kind="ExternalInput")
    rep = nc.dram_tensor("rep", [E, P], F32, kind="ExternalInput")
    idn = nc.dram_tensor("idn", [NB, NB], F32, kind="ExternalInput")
    idnb = nc.dram_tensor("idnb", [P, P], BF16, kind="ExternalInput")
    y_out = nc.dram_tensor("y_out", [H, T], BF16, kind="ExternalOutput")

    x_k = xt_in.ap().rearrange("(k p) t -> k p t", p=P)
    y_k = y_out.ap().rearrange("(k p) t -> k p t", p=P)

    with tile.TileContext(nc) as tc:
        with (
            tc.tile_pool(name="wpool", bufs=1) as wpool,
            tc.tile_pool(name="xpool", bufs=1) as xpool,
            tc.tile_pool(name="ypool", bufs=1) as ypool,
            tc.tile_pool(name="p2pool", bufs=1) as p2pool,
            tc.tile_pool(name="gpool", bufs=1) as gpool,
            tc.tile_pool(name="p1_ps", bufs=1, space="PSUM") as p1_ps,
            tc.tile_pool(name="w_ps", bufs=4, space="PSUM") as w_ps,
        ):
            # ---- big loads on the SWDGE/gpsimd queue, in consumption order:
            # d first (MM1 lhsT), u mid-stream (needed only at MM2 start).
            # Keeping weights on the same queue as x avoids the v2 failure
            # mode where the sync HWDGE weight loads starved at ~60 GB/s
            # behind the x stream and delayed MM1 by 7us.
            d_sb = wpool.tile([P, NK * P], BF16, tag="d")
            nc.gpsimd.dma_start(d_sb[:], d_t.ap())
            xk = []
            for k in range(NK):
                xt = xpool.tile([P, T], BF16, tag=f"x{k}")
                xk.append(xt)
            u_sb = wpool.tile([P, H], BF16, tag="u")
            for k in range(4):
                nc.gpsimd.dma_start(xk[k][:], x_k[k])
            nc.gpsimd.dma_start(u_sb[:], u_in.ap())
            for k in range(4, NK):
                nc.gpsimd.dma_start(xk[k][:], x_k[k])

            # ---- small constants on the sync HWDGE queue ----
            cls_nat = gpool.tile([NB, H], F32, tag="cls")
            nc.sync.dma_start(cls_nat[:], cls_in.ap())
            id_sb = wpool.tile([NB, NB], F32, tag="idn")
            nc.sync.dma_start(id_sb[:], idn.ap())
            rwt_sb = wpool.tile([P, NK * E], F32, tag="rwt")
            nc.sync.dma_start(rwt_sb[:], rwt.ap())
            rep_sb = wpool.tile([E, P], F32, tag="rep")
            nc.sync.dma_start(rep_sb[:], rep.ap())
            idb_sb = wpool.tile([P, P], BF16, tag="idnb")
            nc.sync.dma_start(idb_sb[:], idnb.ap())

            # ---- gates prologue: clean logits -> exact top-2 softmax ----
            clsT = gpool.tile([P, NK * NB], F32, tag="clsT")
            cps = w_ps.tile([P, L], F32, tag="w")
            for k in range(NK):
                nc.tensor.transpose(
                    cps[:, k * NB : (k + 1) * NB],
                    cls_nat[:, k * P : (k + 1) * P],
                    id_sb[:],
                )
            nc.vector.tensor_copy(clsT[:, 0 : NK * NB], cps[:, 0 : NK * NB])

            lg_ps = w_ps.tile([P, L], F32, tag="w")
            for k in range(NK):
                nc.tensor.matmul(
                    lg_ps[0:NB, 0:E],
                    clsT[:, k * NB : (k + 1) * NB],
                    rwt_sb[:, k * E : (k + 1) * E],
                    start=(k == 0),
                    stop=(k == NK - 1),
                )
            lg = gpool.tile([NB, E], F32, tag="lg")
            nc.vector.tensor_copy(lg[:], lg_ps[0:NB, 0:E])

            # top-2 softmax per row (E=8 along free dim)
            m1 = gpool.tile([NB, 1], F32, tag="m1")
            nc.vector.reduce_max(m1[:], lg[:], axis=mybir.AxisListType.X)
            t_sb = gpool.tile([NB, E], F32, tag="t")
            nc.vector.tensor_scalar(
                t_sb[:], lg[:], m1[:], None, op0=mybir.AluOpType.subtract
            )
            # pen = (t >= 0) * 1e30  (knocks out the argmax)
            pen = gpool.tile([NB, E], F32, tag="pen")
            nc.vector.tensor_scalar(
                pen[:], t_sb[:], 0.0, 1e30,
                op0=mybir.AluOpType.is_ge, op1=mybir.AluOpType.mult,
            )
            t2 = gpool.tile([NB, E], F32, tag="t2")
            nc.vector.tensor_sub(t2[:], t_sb[:], pen[:])
            m2 = gpool.tile([NB, 1], F32, tag="m2")
            nc.vector.reduce_max(m2[:], t2[:], axis=mybir.AxisListType.X)
            keep = gpool.tile([NB, E], F32, tag="keep")
            nc.vector.tensor_scalar(
                keep[:], t_sb[:], m2[:], None, op0=mybir.AluOpType.is_ge
            )
            ex = gpool.tile([NB, E], F32, tag="ex")
            nc.scalar.activation(ex[:], t_sb[:], mybir.ActivationFunctionType.Exp)
            eg = gpool.tile([NB, E], F32, tag="eg")
            nc.vector.tensor_mul(eg[:], ex[:], keep[:])
            s_sb = gpool.tile([NB, 1], F32, tag="s")
            nc.vector.reduce_sum(s_sb[:], eg[:], axis=mybir.AxisListType.X)
            rs = gpool.tile([NB, 1], F32, tag="rs")
            nc.vector.reciprocal(rs[:], s_sb[:])
            gts = gpool.tile([NB, E], F32, tag="gts")
            nc.vector.tensor_scalar(
                gts[:], eg[:], rs[:], None, op0=mybir.AluOpType.mult
            )

            # gatesT then replicate x16 along partitions -> gvec [128, NB]
            gt_ps = w_ps.tile([P, L], F32, tag="w")
            nc.tensor.transpose(gt_ps[0:E, 0:NB], gts[:], id_sb[:])
            gtT = gpool.tile([E, NB], F32, tag="gtT")
            nc.vector.tensor_copy(gtT[:], gt_ps[0:E, 0:NB])
            gv_ps = w_ps.tile([P, L], F32, tag="w")
            nc.tensor.matmul(gv_ps[:, 0:NB], rep_sb[:], gtT[:])
            gvec = gpool.tile([P, NB], F32, tag="gvec")
            nc.vector.tensor_copy(gvec[:], gv_ps[:, 0:NB])

            # ---- MM1, k-major so it streams behind the x loads ----
            p1 = [
                p1_ps.tile([P, L], F32, tag=f"p1_{c}", name=f"p1_{c}")
                for c in range(NB)
            ]
            for k in range(NK):
                for c in range(NB):
                    nc.tensor.matmul(
                        p1[c][:],
                        d_sb[:, k * P : (k + 1) * P],
                        xk[k][:, c * L : (c + 1) * L],
                        start=(k == 0),
                        stop=(k == NK - 1),
                    )

            # gate scaling: p2[er, t] = gvec[er, c] * p1[er, t]
            # Split across DVE (tensor_scalar) and Act (copy with per-
            # partition scale) so the post-MM1 pivot latency halves.
            p2 = []
            for c in range(NB):
                p2t = p2pool.tile([P, L], BF16, tag=f"p2_{c}")
                if c % 2 == 0:
                    nc.vector.tensor_scalar(
                        p2t[:], p1[c][:], gvec[:, c : c + 1], None,
                        op0=mybir.AluOpType.mult,
                    )
                else:
                    nc.scalar.activation(
                        p2t[:], p1[c][:],
                        mybir.ActivationFunctionType.Copy,
                        scale=gvec[:, c : c + 1],
                    )
                p2.append(p2t)

            # ---- MM2 + residual, k-major; store each finished yT k-tile ----
            # The PSUM drain (~0.6us per [128,512] tile) would rate-limit
            # everything on one engine, so it is split: chunks 0/1 drain on
            # DVE as fused add(w, x); for chunks 2/3 the PE first accumulates
            # x into PSUM via an identity matmul so the Act engine can drain
            # with a pure copy (Act has no tensor+tensor op; Pool has no PSUM
            # access at all).
            for k in range(NK):
                yt = ypool.tile([P, T], BF16, tag=f"y{k}")
                for c in range(NB):
                    wp = w_ps.tile([P, L], F32, tag="w")
                    if c >= 2:
                        nc.tensor.matmul(
                            wp[:],
                            idb_sb[:],
                            xk[k][:, c * L : (c + 1) * L],
                            start=True,
                            stop=False,
                        )
                    nc.tensor.matmul(
                        wp[:],
                        u_sb[:, k * P : (k + 1) * P],
                        p2[c][:],
                        start=(c < 2),
                        stop=True,
                    )
                    if c < 2:
                        nc.vector.tensor_add(
                            yt[:, c * L : (c + 1) * L],
                            wp[:],
                            xk[k][:, c * L : (c + 1) * L],
                        )
                    else:
                        nc.scalar.activation(
                            yt[:, c * L : (c + 1) * L],
                            wp[:],
                            mybir.ActivationFunctionType.Copy,
                        )
                eng = nc.scalar if k % 2 == 0 else nc.sync
                eng.dma_start(y_k[k], yt[:])

    nc.compile()
    return nc


def _weights_maps(router_w, lora_down, lora_up):
    # D_all[(e,r), h] stacked; lhsT tiles need [p, k, m] = D_all[m, k*128+p]
    d_all = lora_down.reshape(E * R, H)                       # [128, 1024]
    d_t = np.ascontiguousarray(
        d_all.T.reshape(NK, P, E * R).transpose(1, 0, 2).reshape(P, NK * P)
    ).astype(BF16_NP)
    # U_all[(e,r), h] = lora_up[e, h, r]
    u_np = np.ascontiguousarray(
        lora_up.transpose(0, 2, 1).reshape(E * R, H)
    ).astype(BF16_NP)
    # router_wT tiles [p, k, e] = router_w[e, k*128+p]
    rwt_np = np.ascontiguousarray(
        router_w.T.reshape(NK, P, E).transpose(1, 0, 2).reshape(P, NK * E)
    ).astype(np.float32)
    rep_np = np.zeros((E, P), np.float32)
    for e in range(E):
        rep_np[e, e * R : (e + 1) * R] = 1.0
    idn_np = np.eye(NB, dtype=np.float32)
    idnb_np = np.eye(P, dtype=np.float32).astype(BF16_NP)
    return {
        "d_t": d_t, "u_in": u_np, "rwt": rwt_np, "rep": rep_np,
        "idn": idn_np, "idnb": idnb_np,
    }


def get_compiled():
    global _COMPILED
    if _COMPILED is None:
        _COMPILED = _build()
    return _COMPILED


def make_in_maps(x, router_w, lora_down, lora_up):
    x = np.asarray(x, np.float32)
    w_maps = _weights_maps(
        np.asarray(router_w, np.float32),
        np.asarray(lora_down, np.float32),
        np.asarray(lora_up, np.float32),
    )
    in_maps = []
    for i in range(N_CORES):
        shard = x[i * NB : (i + 1) * NB]                       # [NB, L, H]
        xt = np.ascontiguousarray(
            shard.reshape(T, H).T
        ).astype(BF16_NP)                                      # [H, T]
        cls_shard = np.ascontiguousarray(shard[:, 0, :])
        in_maps.append({"xt_in": xt, "cls_in": cls_shard, **w_maps})
    return in_maps


def unshard_one(yt):
    """Per-core device output [H, T] -> [NB, L, H] float32."""
    return np.asarray(yt, np.float32).T.reshape(NB, L, H)


def kernel(x, router_w, lora_down, lora_up):
    nc = get_compiled()
    in_maps = make_in_maps(x, router_w, lora_down, lora_up)
    res = run_bass_kernel_spmd(nc, in_maps, core_ids=list(range(N_CORES)))
    out = np.empty((B, L, H), np.float32)
    for i in range(N_CORES):
        out[i * NB : (i + 1) * NB] = unshard_one(res.results[i]["y_out"])
    return out


# revision 14
# speedup vs baseline: 1.2371x; 1.0762x over previous
"""MoE LoRA adapter layer (top-2 routed, E=8 experts, R=16) on 8 TRN2 NeuronCores.

Strategy: data-parallel over batch B=32 -> 4 batches/core; router + LoRA
weights replicated (tiny). Key observation: E*R = 128 = partition width, so
the per-expert LoRA down/up projections stack into two dense matmuls with the
expert sum folded into the contraction; gates (exactly 0 off the top-2) are
applied by scaling P1 columns.

v2: x is shipped PRE-TRANSPOSED from the host as xT [H, T] bf16, so
  MM1:  p1[er, t]  = sum_h D_all[er, h] * xT[h, t]     (lhsT = D^T tiles)
  MM2:  yT[h, t]   = xT[h, t] + sum_er U_all[er, h] * g*p1[er, t]
needs ZERO on-device transposes: MM2's lhsT is U_all in its natural [er, h]
layout and the output is produced directly in the same transposed layout as
the input, so the residual add reuses the resident xT tiles. The host
transposes the output back. This removes the 128 PE transposes + full-tensor
Scalar copies of v1 and lets stores stream k-tile by k-tile.

DMA: 8 x-tile loads of 512 KiB (4 KiB/partition contiguous runs) on the
SWDGE/gpsimd queue; weights on sync; stores alternate scalar/sync HWDGE
queues. MM1 runs k-major so it streams behind the loads; all four batch
chunks' PSUM accumulators live simultaneously (4 banks) + 4 banks for MM2.
"""

import sys

if "/opt/trn_rl_repo" not in sys.path:
    sys.path.insert(0, "/opt/trn_rl_repo")

import numpy as np
import ml_dtypes

import concourse.bass as bass
import concourse.tile as tile
from concourse import bacc, mybir
from concourse.bass_utils import run_bass_kernel_spmd

B, L, H = 32, 512, 1024
E, R, TOP_K = 8, 16, 2
N_CORES = 8
NB = B // N_CORES          # batches per core = 4
T = NB * L                 # tokens per core = 2048
P = 128                    # partitions
NK = H // P                # H k-tiles = 8

F32 = mybir.dt.float32
BF16 = mybir.dt.bfloat16
BF16_NP = ml_dtypes.bfloat16

_COMPILED = None


def _build():
    """Build + compile the single-core program (same on all 8 cores)."""
    nc = bacc.Bacc("TRN2", target_bir_lowering=False, debug=False)

    xt_in = nc.dram_tensor("xt_in", [H, T], BF16, kind="ExternalInput")
    cls_in = nc.dram_tensor("cls_in", [NB, H], F32, kind="ExternalInput")
    d_t = nc.dram_tensor("d_t", [P, NK * P], BF16, kind="ExternalInput")
    u_in = nc.dram_tensor("u_in", [P, H], BF16, kind="ExternalInput")
    rwt = nc.dram_tensor("rwt", [P, NK * E], F32, kind="ExternalInput")
    rep = nc.dram_tensor("rep", [E, P], F32, kind="ExternalInput")
    idn = nc.dram_tensor("idn", [NB, NB], F32, kind="ExternalInput")
    idnb = nc.dram_tensor("idnb", [P, P], BF16, kind="ExternalInput")
    y_out = nc.dram_tensor("y_out", [H, T], BF16, kind="ExternalOutput")

    x_k = xt_in.ap().rearrange("(k p) t -> k p t", p=P)
    y_k = y_out.ap().rearrange("(k p) t -> k p t", p=P)

    with tile.TileContext(nc) as tc:
        with (
            tc.tile_pool(name="wpool", bufs=1) as wpool,
            tc.tile_pool(name="xpool", bufs=1) as xpool,
            tc.tile_pool(name="ypool", bufs=1) as ypool,
            tc.tile_pool(name="p2pool", bufs=1) as p2pool,
            tc.tile_pool(name="gpool", bufs=1) as gpool,
            tc.tile_pool(name="p1_ps", bufs=1, space="PSUM") as p1_ps,
            tc.tile_pool(name="w_ps", bufs=4, space="PSUM") as w_ps,
        ):
            # ---- big loads on the SWDGE/gpsimd queue, in consumption order:
            # d first (MM1 lhsT), u mid-stream (needed only at MM2 start).
            # Keeping weights on the same queue as x avoids the v2 failure
            # mode where the sync HWDGE weight loads starved at ~60 GB/s
            # behind the x stream and delayed MM1 by 7us.
            d_sb = wpool.tile([P, NK * P], BF16, tag="d")
            nc.gpsimd.dma_start(d_sb[:], d_t.ap())
            xk = []
            for k in range(NK):
                xt = xpool.tile([P, T], BF16, tag=f"x{k}")
                xk.append(xt)
            u_sb = wpool.tile([P, H], BF16, tag="u")
            for k in range(4):
                nc.gpsimd.dma_start(xk[k][:], x_k[k])
            nc.gpsimd.dma_start(u_sb[:], u_in.ap())
            for k in range(4, NK):
                nc.gpsimd.dma_start(xk[k][:], x_k[k])

            # ---- small constants on the sync HWDGE queue ----
            cls_nat = gpool.tile([NB, H], F32, tag="cls")
            nc.sync.dma_start(cls_nat[:], cls_in.ap())
            id_sb = wpool.tile([NB, NB], F32, tag="idn")
            nc.sync.dma_start(id_sb[:], idn.ap())
            rwt_sb = wpool.tile([P, NK * E], F32, tag="rwt")
            nc.sync.dma_start(rwt_sb[:], rwt.ap())
            rep_sb = wpool.tile([E, P], F32, tag="rep")
            nc.sync.dma_start(rep_sb[:], rep.ap())
            idb_sb = wpool.tile([P, P], BF16, tag="idnb")
            nc.sync.dma_start(idb_sb[:], idnb.ap())

            # ---- gates prologue: clean logits -> exact top-2 softmax ----
            clsT = gpool.tile([P, NK * NB], F32, tag="clsT")
            cps = w_ps.tile([P, L], F32, tag="w")
            for k in range(NK):
                nc.tensor.transpose(
                    cps[:, k * NB : (k + 1) * NB],
                    cls_nat[:, k * P : (k + 1) * P],
                    id_sb[:],
                )
            nc.vector.tensor_copy(clsT[:, 0 : NK * NB], cps[:, 0 : NK * NB])

            lg_ps = w_ps.tile([P, L], F32, tag="w")
            for k in range(NK):
                nc.tensor.matmul(
                    lg_ps[0:NB, 0:E],
                    clsT[:, k * NB : (k + 1) * NB],
                    rwt_sb[:, k * E : (k + 1) * E],
                    start=(k == 0),
                    stop=(k == NK - 1),
                )
            lg = gpool.tile([NB, E], F32, tag="lg")
            nc.vector.tensor_copy(lg[:], lg_ps[0:NB, 0:E])

            # top-2 softmax per row (E=8 along free dim)
            m1 = gpool.tile([NB, 1], F32, tag="m1")
            nc.vector.reduce_max(m1[:], lg[:], axis=mybir.AxisListType.X)
            t_sb = gpool.tile([NB, E], F32, tag="t")
            nc.vector.tensor_scalar(
                t_sb[:], lg[:], m1[:], None, op0=mybir.AluOpType.subtract
            )
            # pen = (t >= 0) * 1e30  (knocks out the argmax)
            pen = gpool.tile([NB, E], F32, tag="pen")
            nc.vector.tensor_scalar(
                pen[:], t_sb[:], 0.0, 1e30,
                op0=mybir.AluOpType.is_ge, op1=mybir.AluOpType.mult,
            )
            t2 = gpool.tile([NB, E], F32, tag="t2")
            nc.vector.tensor_sub(t2[:], t_sb[:], pen[:])
            m2 = gpool.tile([NB, 1], F32, tag="m2")
            nc.vector.reduce_max(m2[:], t2[:], axis=mybir.AxisListType.X)
            keep = gpool.tile([NB, E], F32, tag="keep")
            nc.vector.tensor_scalar(
                keep[:], t_sb[:], m2[:], None, op0=mybir.AluOpType.is_ge
            )
            ex = gpool.tile([NB, E], F32, tag="ex")
            nc.scalar.activation(ex[:], t_sb[:], mybir.ActivationFunctionType.Exp)
            eg = gpool.tile([NB, E], F32, tag="eg")
            nc.vector.tensor_mul(eg[:], ex[:], keep[:])
            s_sb = gpool.tile([NB, 1], F32, tag="s")
            nc.vector.reduce_sum(s_sb[:], eg[:], axis=mybir.AxisListType.X)
            rs = gpool.tile([NB, 1], F32, tag="rs")
            nc.vector.reciprocal(rs[:], s_sb[:])
            gts = gpool.tile([NB, E], F32, tag="gts")
            nc.vector.tensor_scalar(
                gts[:], eg[:], rs[:], None, op0=mybir.AluOpType.mult
            )

            # gatesT then replicate x16 along partitions -> gvec [128, NB]
            gt_ps = w_ps.tile([P, L], F32, tag="w")
            nc.tensor.transpose(gt_ps[0:E, 0:NB], gts[:], id_sb[:])
            gtT = gpool.tile([E, NB], F32, tag="gtT")
            nc.vector.tensor_copy(gtT[:], gt_ps[0:E, 0:NB])
            gv_ps = w_ps.tile([P, L], F32, tag="w")
            nc.tensor.matmul(gv_ps[:, 0:NB], rep_sb[:], gtT[:])
            gvec = gpool.tile([P, NB], F32, tag="gvec")
            nc.vector.tensor_copy(gvec[:], gv_ps[:, 0:NB])

            # ---- MM1, k-major so it streams behind the x loads ----
            p1 = [
                p1_ps.tile([P, L], F32, tag=f"p1_{c}", name=f"p1_{c}")
                for c in range(NB)
            ]
            for k in range(NK):
                for c in range(NB):
                    nc.tensor.matmul(
                        p1[c][:],
                        d_sb[:, k * P : (k + 1) * P],
                        xk[k][:, c * L : (c + 1) * L],
                        start=(k == 0),
                        stop=(k == NK - 1),
                    )

            # gate scaling: p2[er, t] = gvec[er, c] * p1[er, t]
            # Split across DVE (tensor_scalar) and Act (copy with per-
            # partition scale) so the post-MM1 pivot latency halves.
            p2 = []
            for c in range(NB):
                p2t = p2pool.tile([P, L], BF16, tag=f"p2_{c}")
                if c % 2 == 0:
                    nc.vector.tensor_scalar(
                        p2t[:], p1[c][:], gvec[:, c : c + 1], None,
                        op0=mybir.AluOpType.mult,
                    )
                else:
                    nc.scalar.activation(
                        p2t[:], p1[c][:],
                        mybir.ActivationFunctionType.Copy,
                        scale=gvec[:, c : c + 1],
                    )
                p2.append(p2t)

            # ---- MM2 + residual, k-major; store each finished yT k-tile ----
            # The PSUM drain (~0.6us per [128,512] tile) would rate-limit
            # everything on one engine, so it is split: chunks 0/1 drain on
            # DVE as fused add(w, x); for chunks 2/3 the PE first accumulates
            # x into PSUM via an identity matmul so the Act engine can drain
            # with a pure copy (Act has no tensor+tensor op; Pool has no PSUM
            # access at all).
            for k in range(NK):
                yt = ypool.tile([P, T], BF16, tag=f"y{k}")
                for c in range(NB):
                    wp = w_ps.tile([P, L], F32, tag="w")
                    if c >= 2:
                        nc.tensor.matmul(
                            wp[:],
                            idb_sb[:],
                            xk[k][:, c * L : (c + 1) * L],
                            start=True,
                            stop=False,
                        )
                    nc.tensor.matmul(
                        wp[:],
                        u_sb[:, k * P : (k + 1) * P],
                        p2[c][:],
                        start=(c < 2),
                        stop=True,
                    )
                    if c < 2:
                        nc.vector.tensor_add(
                            yt[:, c * L : (c + 1) * L],
                            wp[:],
                            xk[k][:, c * L : (c + 1) * L],
                        )
                    else:
                        nc.scalar.activation(
                            yt[:, c * L : (c + 1) * L],
                            wp[:],
                            mybir.ActivationFunctionType.Copy,
                        )
                # store triggers cost ~650ns on the issuing queue; sync and
                # gpsimd are idle here (scalar/vector are busy draining PSUM)
                eng = nc.sync if k % 2 == 0 else nc.gpsimd
                eng.dma_start(y_k[k], yt[:])

    nc.compile()
    return nc


def _weights_maps(router_w, lora_down, lora_up):
    # D_all[(e,r), h] stacked; lhsT tiles need [p, k, m] = D_all[m, k*128+p]
    d_all = lora_down.reshape(E * R, H)                       # [128, 1024]
    d_t = np.ascontiguousarray(
        d_all.T.reshape(NK, P, E * R).transpose(1, 0, 2).reshape(P, NK * P)
    ).astype(BF16_NP)
    # U_all[(e,r), h] = lora_up[e, h, r]
    u_np = np.ascontiguousarray(
        lora_up.transpose(0, 2, 1).reshape(E * R, H)
    ).astype(BF16_NP)
    # router_wT tiles [p, k, e] = router_w[e, k*128+p]
    rwt_np = np.ascontiguousarray(
        router_w.T.reshape(NK, P, E).transpose(1, 0, 2).reshape(P, NK * E)
    ).astype(np.float32)
    rep_np = np.zeros((E, P), np.float32)
    for e in range(E):
        rep_np[e, e * R : (e + 1) * R] = 1.0
    idn_np = np.eye(NB, dtype=np.float32)
    idnb_np = np.eye(P, dtype=np.float32).astype(BF16_NP)
    return {
        "d_t": d_t, "u_in": u_np, "rwt": rwt_np, "rep": rep_np,
        "idn": idn_np, "idnb": idnb_np,
    }


def get_compiled():
    global _COMPILED
    if _COMPILED is None:
        _COMPILED = _build()
    return _COMPILED


def make_in_maps(x, router_w, lora_down, lora_up):
    x = np.asarray(x, np.float32)
    w_maps = _weights_maps(
        np.asarray(router_w, np.float32),
        np.asarray(lora_down, np.float32),
        np.asarray(lora_up, np.float32),
    )
    in_maps = []
    for i in range(N_CORES):
        shard = x[i * NB : (i + 1) * NB]                       # [NB, L, H]
        xt = np.ascontiguousarray(
            shard.reshape(T, H).T
        ).astype(BF16_NP)                                      # [H, T]
        cls_shard = np.ascontiguousarray(shard[:, 0, :])
        in_maps.append({"xt_in": xt, "cls_in": cls_shard, **w_maps})
    return in_maps


def unshard_one(yt):
    """Per-core device output [H, T] -> [NB, L, H] float32."""
    return np.asarray(yt, np.float32).T.reshape(NB, L, H)


def kernel(x, router_w, lora_down, lora_up):
    nc = get_compiled()
    in_maps = make_in_maps(x, router_w, lora_down, lora_up)
    res = run_bass_kernel_spmd(nc, in_maps, core_ids=list(range(N_CORES)))
    out = np.empty((B, L, H), np.float32)
    for i in range(N_CORES):
        out[i * NB : (i + 1) * NB] = unshard_one(res.results[i]["y_out"])
    return out
